# revision 53
# baseline (speedup 1.0000x reference)
"""Distributed Trainium2 attention-block kernel (8 NeuronCores).

Problem: y = LN(x) -> QKV -> 16-head attention (seq 2048, dh 64) -> out-proj.
x [2,2048,1024] f32.

Sharding: token-parallel. Core c handles batch c//4, token quarter c%4
(512 query tokens). Each core computes Q,K,V for its own 512 tokens
(all heads), AllGathers K^T and augmented V within its 4-core batch
group (bf16), then runs attention for its 512 queries over the full
sequence and the final projection. Output shards are disjoint -> no
reduction.

Weights are cast to bf16 on the host (numpy) before upload: halves the
HBM weight traffic (the AllGathers contend with it for bandwidth) and
runs every projection at full bf16 PE rate with hidden LDWEIGHTS.

Collective schedule: K/V for each head-group are projected first
(k0 -> AG, v0 -> AG, q(g0), k1 -> AG, v1 -> AG, q(g1)) so all four
AGs are triggered in the first ~50us and overlap the remaining
projections + local attention. AG order k0,v0,k1,v1 keeps remote-pass
PV fed (v0 must land right after k0). AG buffers are laid out with the
partition dim (dh rows / token rows) as the concatenation axis, so
each rank's gathered chunk reads back as one contiguous 128-row block
with 4KB-per-partition DMA descriptors (the [rank*512-row, token]
layout produced 1KB partition-strided gathers at ~3GB/s/engine and
stalled the remote passes ~50us).

Attention per head: dots computed transposed (k on partitions, q free)
so exp'd probabilities feed PV directly as the moving operand; PV's
stationary is [V_tile | ones] (M=65) so the softmax denominator
accumulates in PSUM row 64 for free. Softmax skips max-subtraction
(scaled dots ~N(0,1) by construction). exp reads dots PSUM in batches
of 3 k-tiles. Local passes (SBUF-resident chunk) run for all 16 heads
before the remote passes consume the gathered buffers at rank-dynamic
row offsets.
"""

import os
import numpy as np
import ml_dtypes

import concourse.bass as bass
import concourse.tile as tile
from concourse import mybir
from concourse.bass import ds
from concourse.bass_utils import run_bass_kernel_spmd
from concourse.masks import make_identity

F32 = mybir.dt.float32
BF16 = mybir.dt.bfloat16

B, S, D = 2, 2048, 1024
H, DH = 16, 64
T = 512           # query tokens per core
P = 128
NKT = S // P      # 16 k-tiles
LN_EPS = 1e-5
SCALE = DH ** -0.5
EXP_BATCH = 2     # k-tiles per exp ACTIVATE call

_MAXW = 1


def _split_multiwaits(nc):
    """This container's walrus rejects >1 sync wait/update per instruction.
    Move extras onto adjacent same-engine NoOps."""
    import bass_rust

    for bb in nc.main_func.blocks:
        new_insts = []
        for inst in bb.instructions:
            si = inst.sync_info
            pre, post = [], []
            if si is not None:
                waits = list(si.on_wait or [])
                ups = list(si.on_update or [])
                if len(waits) > _MAXW or len(ups) > _MAXW:
                    for i in range(_MAXW, len(waits), _MAXW):
                        pre.append(bass_rust.InstNoOp(
                            name=f"I-{nc.next_id()}", engine=inst.engine,
                            ins=[], outs=[],
                            sync_info=mybir.SyncInfo(
                                on_wait=waits[i:i + _MAXW], on_update=[])))
                    for i in range(_MAXW, len(ups), _MAXW):
                        post.append(bass_rust.InstNoOp(
                            name=f"I-{nc.next_id()}", engine=inst.engine,
                            ins=[], outs=[],
                            sync_info=mybir.SyncInfo(
                                on_wait=[], on_update=ups[i:i + _MAXW])))
                    inst.sync_info = mybir.SyncInfo(
                        on_wait=waits[:_MAXW], on_update=ups[:_MAXW])
            new_insts.extend(pre)
            new_insts.append(inst)
            new_insts.extend(post)
        bb.instructions[:] = new_insts


def _maybe_install_ntff_hook():
    """Optional NTFF profiling support (BASS_TRACE=1); harmless if absent."""
    if not os.environ.get("BASS_TRACE"):
        return
    import sys
    import types
    if "antenv.axon_hooks" in sys.modules:
        return
    try:
        mod = types.ModuleType("antenv.axon_hooks")
        _h = [None]
        mod.set_axon_ntff_profile_hook = lambda h: _h.__setitem__(0, h)
        mod.get_axon_ntff_profile_hook = lambda: _h[0]
        import antenv
        from trn_agent_boot.trn_boot import _ntff_profile_via_ctypes
        hook = _ntff_profile_via_ctypes('/opt/axon/libaxon_pjrt.so')
        sys.modules["antenv.axon_hooks"] = mod
        antenv.axon_hooks = mod
        mod.set_axon_ntff_profile_hook(hook)
    except Exception:
        pass


def build(apply_ln_affine, apply_b_out):
    nc = bass.Bass()

    x_ext = nc.declare_dram_parameter("x", [T, D], BF16, isOutput=False)
    gamma_ext = nc.declare_dram_parameter("ln_gamma", [1, D], F32, isOutput=False)
    beta_ext = nc.declare_dram_parameter("ln_beta", [1, D], F32, isOutput=False)
    wqkv_ext = nc.declare_dram_parameter("w_qkv", [D, 3 * D], BF16, isOutput=False)
    wout_ext = nc.declare_dram_parameter("w_out", [D, D], BF16, isOutput=False)
    bout_ext = nc.declare_dram_parameter("b_out", [1, D], F32, isOutput=False)
    out_ext = nc.declare_dram_parameter("out", [T, D], BF16, isOutput=True)

    groups = [[0, 1, 2, 3], [4, 5, 6, 7]]
    NDT = D // P   # 8 contraction tiles over model dim
    NTT = T // P   # 4 token tiles per core
    NHP = H // 2   # 8 head pairs
    VA = 2 * 65    # augmented-v columns per head pair

    from contextlib import ExitStack
    with tile.TileContext(nc) as tc, ExitStack() as stack:
        consts = stack.enter_context(tc.tile_pool(name="consts", bufs=1))
        sb_main = stack.enter_context(tc.tile_pool(name="sb_main", bufs=1))
        p23 = stack.enter_context(tc.tile_pool(name="p23", bufs=1))

        eps_t = consts.tile([P, 1], F32)
        nc.vector.memset(eps_t, LN_EPS)
        ones8 = consts.tile([P, 8], F32)
        nc.vector.memset(ones8, 1.0)

        if apply_ln_affine:
            gammaB = consts.tile([P, D], F32)
            betaB = consts.tile([P, D], F32)
            nc.sync.dma_start(out=gammaB, in_=bass.AP(
                tensor=gamma_ext.tensor, offset=gamma_ext.offset,
                ap=[[0, P]] + gamma_ext.ap[1:]))
            nc.sync.dma_start(out=betaB, in_=bass.AP(
                tensor=beta_ext.tensor, offset=beta_ext.offset,
                ap=[[0, P]] + beta_ext.ap[1:]))
        if apply_b_out:
            boutB = consts.tile([P, D], F32)
            nc.sync.dma_start(out=boutB, in_=bass.AP(
                tensor=bout_ext.tensor, offset=bout_ext.offset,
                ap=[[0, P]] + bout_ext.ap[1:]))

        # persistent activations (all bf16)
        xnT = [sb_main.tile([P, T], BF16, tag=f"xnT{i}", name=f"xnT{i}")
               for i in range(NDT)]
        qT = [sb_main.tile([P, T], BF16, tag=f"qT{i}", name=f"qT{i}")
              for i in range(NHP)]
        attnT = [sb_main.tile([P, T], BF16, tag=f"attnT{i}", name=f"attnT{i}")
                 for i in range(NHP)]
        wout_sb = [sb_main.tile([P, D], BF16, tag=f"wout{i}", name=f"wout{i}")
                   for i in range(NDT)]
        # local K^T / augmented-V (this core's token chunk), kept resident
        kt_l = [p23.tile([P, T], BF16, tag=f"ktl{i}", name=f"ktl{i}")
                for i in range(NHP)]                  # i = 4*g + hq
        v_l = [p23.tile([P, 4 * VA], BF16, tag=f"vl{i}", name=f"vl{i}")
               for i in range(8)]                     # i = 4*g + token tile

        # AG buffers (internal DRAM), bf16, split by head-pair-half.
        # Partition dim (128) is the rank-concat axis: each rank's chunk
        # reads back as one contiguous 128-row block (4KB/partition).
        k_in2 = [nc.dram_tensor(f"k_in{g}", [P, 4 * T], BF16).ap()
                 for g in range(2)]
        k_out2 = [nc.dram_tensor(f"k_out{g}", [4 * P, 4 * T], BF16).ap()
                  for g in range(2)]
        v_in2 = [nc.dram_tensor(f"v_in{g}", [P, 4 * (4 * VA)], BF16).ap()
                 for g in range(2)]
        v_out2 = [nc.dram_tensor(f"v_out{g}", [4 * P, 4 * (4 * VA)], BF16).ap()
                  for g in range(2)]
        recip_d = nc.dram_tensor("recip_d", [H, T], F32).ap()

        # ---------------- Phase 1: LayerNorm + transpose ----------------
        with tc.tile_pool(name="p1sb", bufs=3) as p1sb, \
             tc.tile_pool(name="p1ps", bufs=4, space="PSUM") as p1ps:
            ident = p1sb.tile([P, P], BF16, tag="ident", bufs=1)
            make_identity(nc, ident)
            for tt in range(NTT):
                x_t = p1sb.tile([P, D], BF16, tag="x")
                nc.sync.dma_start(out=x_t, in_=x_ext[tt * P:(tt + 1) * P, :])
                stats = p1sb.tile([P, 2, nc.vector.BN_STATS_DIM], F32, tag="st")
                for sg in range(2):
                    nc.vector.bn_stats(out=stats[:, sg, :],
                                       in_=x_t[:, sg * 512:(sg + 1) * 512])
                mv = p1sb.tile([P, nc.vector.BN_AGGR_DIM], F32, tag="mv")
                nc.vector.bn_aggr(out=mv, in_=stats)
                rstd = p1sb.tile([P, 1], F32, tag="rstd")
                nc.scalar.activation(out=rstd, in_=mv[:, 1:2],
                                     func=mybir.ActivationFunctionType.Sqrt,
                                     bias=eps_t, scale=1.0)
                nc.vector.reciprocal(out=rstd, in_=rstd)
                xn_t = p1sb.tile([P, D], BF16, tag="xn")
                nc.vector.tensor_scalar(
                    out=xn_t, in0=x_t, scalar1=mv[:, 0:1], scalar2=rstd,
                    op0=mybir.AluOpType.subtract, op1=mybir.AluOpType.mult)
                if apply_ln_affine:
                    nc.vector.tensor_mul(out=xn_t, in0=xn_t, in1=gammaB)
                    nc.vector.tensor_add(out=xn_t, in0=xn_t, in1=betaB)
                for dt in range(NDT):
                    ps_tr = p1ps.tile([P, P], BF16, tag="tr")
                    nc.tensor.transpose(ps_tr, xn_t[:, dt * P:(dt + 1) * P],
                                        ident)
                    nc.vector.tensor_copy(out=xnT[dt][:, tt * P:(tt + 1) * P],
                                          in_=ps_tr)

        # ---------------- Phase 2: QKV projection + AllGathers ----------------
        with tc.tile_pool(name="p2c", bufs=2) as p2c, \
             tc.tile_pool(name="p2sb", bufs=4) as p2sb, \
             tc.tile_pool(name="p2ps", bufs=4, space="PSUM") as p2ps:
            wq_view = wqkv_ext.rearrange("(dt p) f -> dt p f", p=P)

            def load_col_chunk(base, tag):
                w_c = p2c.tile([P, NDT, T], BF16, tag="wcol", name=tag)
                nc.sync.dma_start(
                    out=w_c,
                    in_=wq_view[:, :, base:base + T]
                    .rearrange("dt p f -> p dt f"))
                return w_c

            def proj_colT(lhsT_of_dt, dst):
                ps = p2ps.tile([P, T], F32, tag="pqk")
                for dt in range(NDT):
                    nc.tensor.matmul(ps, lhsT_of_dt(dt), xnT[dt],
                                     start=(dt == 0), stop=(dt == NDT - 1))
                nc.vector.tensor_copy(out=dst, in_=ps)

            def proj_k_group(g, kc):
                for i in range(4):
                    proj_colT(lambda dt, i=i: kc[:, dt, i * P:(i + 1) * P],
                              kt_l[4 * g + i])
                    nc.sync.dma_start(
                        out=k_in2[g][:, i * T:(i + 1) * T],
                        in_=kt_l[4 * g + i])
                nc.gpsimd.collective_compute(
                    "AllGather", mybir.AluOpType.bypass,
                    replica_groups=groups,
                    ins=[k_in2[g].opt()], outs=[k_out2[g].opt()])

            def proj_v_group(g, vc):
                for vt_i in range(NTT):
                    ps = p2ps.tile([P, T], F32, tag="pv")
                    for dt in range(NDT):
                        nc.tensor.matmul(
                            ps, xnT[dt][:, vt_i * P:(vt_i + 1) * P],
                            vc[:, dt, :],
                            start=(dt == 0), stop=(dt == NDT - 1))
                    vl = v_l[4 * g + vt_i]
                    nc.vector.tensor_copy(
                        out=vl.rearrange("p (h f) -> p h f", h=8)[:, :, 0:64],
                        in_=ps.rearrange("p (h f) -> p h f", h=8))
                    nc.vector.tensor_copy(
                        out=vl.rearrange("p (h f) -> p h f", h=8)[:, :, 64:65],
                        in_=ones8.rearrange("p (h o) -> p h o", h=8))
                    nc.sync.dma_start(
                        out=v_in2[g][:, vt_i * (4 * VA):(vt_i + 1) * (4 * VA)],
                        in_=vl)
                nc.gpsimd.collective_compute(
                    "AllGather", mybir.AluOpType.bypass, replica_groups=groups,
                    ins=[v_in2[g].opt()], outs=[v_out2[g].opt()])

            kc0 = load_col_chunk(D, "kc0")
            vc0 = load_col_chunk(2 * D, "vc0")
            proj_k_group(0, kc0)          # -> AG k0
            qc0 = load_col_chunk(0, "qc0")
            proj_v_group(0, vc0)          # -> AG v0
            kc1 = load_col_chunk(D + T, "kc1")
            for ct in range(4):           # q head-pairs 0-3
                proj_colT(lambda dt, ct=ct: qc0[:, dt, ct * P:(ct + 1) * P],
                          qT[ct])
            vc1 = load_col_chunk(2 * D + T, "vc1")
            proj_k_group(1, kc1)          # -> AG k1
            qc1 = load_col_chunk(T, "qc1")
            proj_v_group(1, vc1)          # -> AG v1
            for ct in range(4):           # q head-pairs 4-7
                proj_colT(lambda dt, ct=ct: qc1[:, dt, ct * P:(ct + 1) * P],
                          qT[4 + ct])

        # ---------------- Phase 3: attention ----------------
        rem_kts = list(range(4, NKT))
        with tc.tile_pool(name="p3kv", bufs=1) as p3kv, \
             tc.tile_pool(name="p3sb", bufs=4) as p3sb, \
             tc.tile_pool(name="p3o", bufs=1) as p3o, \
             tc.tile_pool(name="p3pt", bufs=18) as p3pt, \
             tc.tile_pool(name="p3po", bufs=2, space="PSUM") as p3po, \
             tc.tile_pool(name="p3pd", bufs=3, space="PSUM") as p3pd:
            o_raw = [p3o.tile([65, T], F32, tag=f"oraw{h}", name=f"oraw{h}")
                     for h in range(H)]
            o_loc = [p3o.tile([65, T], BF16, tag=f"oloc{h}", name=f"oloc{h}")
                     for h in range(H)]
            krem2, vrem2 = {}, {}

            def load_remote(g):
                # all K chunks before any V chunk: V waits on the later
                # v-AllGather, and a blocked V DMA at the queue head would
                # stall the K chunks the dots/exp stream needs first
                eng = nc.sync if g == 0 else nc.gpsimd
                rank4 = eng.partition_id() % 4
                krem, vrem = [], []
                offs = []
                for j in range(3):
                    offs.append(eng.snap(((rank4 + 1 + j) % 4) * P,
                                         min_val=0, max_val=3 * P))
                for j in range(3):
                    kr = p3kv.tile([P, 4, T], BF16, tag=f"kr{j}",
                                   name=f"kr{g}_{j}")
                    eng.dma_start(
                        out=kr,
                        in_=k_out2[g][ds(offs[j], P), :]
                        .rearrange("p (h t) -> p h t", h=4))
                    krem.append(kr)
                for j in range(3):
                    vr = p3kv.tile([P, 4, 4 * VA], BF16, tag=f"vr{j}",
                                   name=f"vr{g}_{j}")
                    eng.dma_start(
                        out=vr,
                        in_=v_out2[g][ds(offs[j], P), :]
                        .rearrange("p (w c) -> p w c", w=4))
                    vrem.append(vr)
                krem2[g], vrem2[g] = krem, vrem

            def k_src(g, hq, kt):
                c, w = kt // 4, kt % 4
                if c == 0:
                    return kt_l[4 * g + hq][:, w * P:(w + 1) * P]
                return krem2[g][c - 1][:, hq, w * P:(w + 1) * P]

            def v_src(g, hq, ab, kt):
                c, w = kt // 4, kt % 4
                base = hq * VA + ab * 65
                if c == 0:
                    return v_l[4 * g + w][:, base:base + 65]
                return vrem2[g][c - 1][:, w, base:base + 65]

            def attn_pass(g, hq, kts_all, drain, split_pv=False):
                """dots->exp->PV over kts_all; drain(ps_o) at end.

                split_pv: emit ALL dots+exp before any PV matmul.  Remote
                passes need this: their PV waits on the v-AllGather, and a
                blocked PV in the serial tensor queue would stall the
                dots/exp stream behind it.  The exp'd P tiles buffer in
                SBUF (p3pt holds a full 12-kt pass)."""
                hp = 4 * g + hq
                batches = [kts_all[i:i + EXP_BATCH]
                           for i in range(0, len(kts_all), EXP_BATCH)]
                first_kt = kts_all[0]
                last_kt = kts_all[-1]
                ps_o = None

                def emit_pv(pkts, ppts, is_last):
                    for i, kt in enumerate(pkts):
                        for ab in range(2):
                            nc.tensor.matmul(
                                ps_o[ab], v_src(g, hq, ab, kt),
                                ppts[ab][:, i, :],
                                start=(kt == first_kt),
                                stop=(is_last and kt == last_kt))

                plist = []
                if not split_pv:
                    ps_o = [p3po.tile([65, T], F32, tag="po",
                                      name=f"po{drain.__name__}{hp}_{ab}")
                            for ab in range(2)]
                for kts in batches:
                    nb = len(kts)
                    pd = [p3pd.tile([P, EXP_BATCH, T], F32, tag="pd",
                                    name=f"pd{drain.__name__}{hp}_{kts[0]}_{ab}")
                          for ab in range(2)]
                    for i, kt in enumerate(kts):
                        for ab in range(2):
                            nc.tensor.matmul(
                                pd[ab][:, i, :],
                                k_src(g, hq, kt)[ab * 64:(ab + 1) * 64, :],
                                qT[hp][ab * 64:(ab + 1) * 64, :],
                                start=True, stop=True,
                                tile_position=(ab * 64, 0))
                    pts = []
                    for ab in range(2):
                        pt = p3pt.tile([P, EXP_BATCH, T], BF16, tag="pt")
                        nc.scalar.activation(
                            out=pt[:, 0:nb, :], in_=pd[ab][:, 0:nb, :],
                            func=mybir.ActivationFunctionType.Exp,
                            scale=SCALE)
                        pts.append(pt)
                    plist.append((list(kts), pts))
                    if not split_pv and len(plist) > 1:
                        emit_pv(*plist.pop(0), False)
                if split_pv:
                    ps_o = [p3po.tile([65, T], F32, tag="po",
                                      name=f"po{drain.__name__}{hp}_{ab}")
                            for ab in range(2)]
                while plist:
                    kp = plist.pop(0)
                    emit_pv(*kp, not plist)
                drain(hp, ps_o)

            def drain_local(hp, ps_o):
                for ab in range(2):
                    nc.vector.tensor_copy(out=o_loc[2 * hp + ab],
                                          in_=ps_o[ab])

            def drain_remote(hp, ps_o):
                for ab in range(2):
                    h = 2 * hp + ab
                    nc.vector.tensor_add(out=o_raw[h], in0=ps_o[ab],
                                         in1=o_loc[h])

            for g in range(2):
                for hq in range(4):
                    attn_pass(g, hq, list(range(4)), drain_local)

            # w_out (bf16) load while local attention runs
            for it in range(NDT):
                nc.sync.dma_start(
                    out=wout_sb[it], in_=wout_ext[it * P:(it + 1) * P, :])

            for g in range(2):
                load_remote(g)
                for hq in range(4):
                    attn_pass(g, hq, rem_kts, drain_remote, split_pv=True)
                # deferred normalization: one reciprocal for the group's 8
                # heads, broadcast across partitions via a DRAM round-trip
                sums_g = p3sb.tile([8, T], F32, tag="sums")
                for j in range(8):
                    h = 8 * g + j
                    nc.sync.dma_start(out=sums_g[j:j + 1, :],
                                      in_=o_raw[h][64:65, :])
                nc.vector.reciprocal(out=sums_g, in_=sums_g)
                nc.sync.dma_start(out=recip_d[8 * g:8 * g + 8, :], in_=sums_g)
                for hq in range(4):
                    hp = 4 * g + hq
                    for ab in range(2):
                        h = 2 * hp + ab
                        recipB = p3sb.tile([64, T], F32, tag="rb")
                        rd = recip_d[h:h + 1, :]
                        nc.sync.dma_start(out=recipB, in_=bass.AP(
                            tensor=rd.tensor, offset=rd.offset,
                            ap=[[0, 64]] + rd.ap[1:]))
                        nc.vector.tensor_mul(
                            out=attnT[hp][ab * 64:(ab + 1) * 64, :],
                            in0=o_raw[h][0:64, :],
                            in1=recipB)

        # ---------------- Phase 4: output projection ----------------
        with tc.tile_pool(name="p4sb", bufs=3) as p4sb, \
             tc.tile_pool(name="p4ps", bufs=2, space="PSUM") as p4ps:
            for tt in range(NTT):
                for dc in range(2):
                    ps_y = p4ps.tile([P, T], F32, tag="py")
                    for it in range(NDT):
                        nc.tensor.matmul(
                            ps_y, attnT[it][:, tt * P:(tt + 1) * P],
                            wout_sb[it][:, dc * T:(dc + 1) * T],
                            start=(it == 0), stop=(it == NDT - 1))
                    y_s = p4sb.tile([P, T], BF16, tag="y")
                    if apply_b_out:
                        nc.vector.tensor_add(
                            out=y_s, in0=ps_y,
                            in1=boutB[:, dc * T:(dc + 1) * T])
                    else:
                        nc.vector.tensor_copy(out=y_s, in_=ps_y)
                    nc.sync.dma_start(
                        out=out_ext[tt * P:(tt + 1) * P,
                                    dc * T:(dc + 1) * T],
                        in_=y_s)

    _split_multiwaits(nc)
    return nc


_CACHE = {}
LAST_RESULTS = None


def kernel(x, ln_gamma, ln_beta, w_qkv, w_out, b_out):
    global LAST_RESULTS
    _maybe_install_ntff_hook()

    x = np.ascontiguousarray(
        np.asarray(x, dtype=np.float32).astype(ml_dtypes.bfloat16))
    ln_gamma = np.asarray(ln_gamma, dtype=np.float32).reshape(1, D)
    ln_beta = np.asarray(ln_beta, dtype=np.float32).reshape(1, D)
    w_qkv_bf = np.ascontiguousarray(
        np.asarray(w_qkv, dtype=np.float32).astype(ml_dtypes.bfloat16))
    w_out_bf = np.ascontiguousarray(
        np.asarray(w_out, dtype=np.float32).astype(ml_dtypes.bfloat16))
    b_out = np.asarray(b_out, dtype=np.float32).reshape(1, D)

    apply_ln_affine = not (np.all(ln_gamma == 1.0) and np.all(ln_beta == 0.0))
    apply_b_out = not np.all(b_out == 0.0)

    key = (apply_ln_affine, apply_b_out)
    if key not in _CACHE:
        _CACHE[key] = build(*key)
    nc = _CACHE[key]

    in_maps = []
    for c in range(8):
        b, t = c // 4, c % 4
        in_maps.append({
            "x": np.ascontiguousarray(x[b, t * T:(t + 1) * T, :]),
            "ln_gamma": ln_gamma,
            "ln_beta": ln_beta,
            "w_qkv": w_qkv_bf,
            "w_out": w_out_bf,
            "b_out": b_out,
        })

    trace = bool(os.environ.get("BASS_TRACE"))
    res = run_bass_kernel_spmd(nc, in_maps, core_ids=list(range(8)),
                               trace=trace)
    LAST_RESULTS = res

    y = np.empty((B, S, D), dtype=np.float32)
    for c in range(8):
        b, t = c // 4, c % 4
        y[b, t * T:(t + 1) * T, :] = np.asarray(
            res.results[c]["out"]).astype(np.float32)
    return y
